# revision 34
# baseline (speedup 1.0000x reference)
"""Trainium2 Bass kernel for nn_Decoder_gru_2_8589935086.

Computes, for all M=3486 unordered pairs (i<j) of the N=84 graph nodes:
GRUCell(x[i], x[j]) -> 3x (Linear -> ReLU -> full-tensor LayerNorm) -> Linear
-> sigmoid, scattered into a symmetric [84, 84] matrix.

Key structural choices (single NeuronCore):
  * Pair expansion commutes with the GRU input/hidden matmuls: compute
    A = [x|1]@[W_ih.T;b_ih], B = [x|1]@[W_hh.T;b_hh] ([84, 192]) once, then
    gather rows per-pair with one-hot selection-matrix matmuls accumulating
    A[iu] + B[ju] directly in PSUM.
  * The M pairs are packed as two halves of F=1743 columns.  A custom pair
    order makes ju IDENTICAL for both halves on columns 0:1722 ("shared-j"):
    for each j, its pairs (i, j) are split half/half between the two lanes.
    The B-side/x2 gathers then need ONE full-width matmul (duplicated
    weights on both partition halves) instead of two, and only one sju
    section needs to be DMA'd: 10 PE streams and 3*cw selection columns
    per chunk instead of 14 and 4*cw.  The 21 leftover columns from
    odd-count groups live in the last chunk, which keeps the generic
    4-section format.
  * Since b2=b3=0 and LayerNorm (with identity affine) is exactly
    scale-invariant, the rsqrt scales of LN1/LN2 never need to be computed:
    work in "hat space" y^ = y/scale.  Only the means m1, m^2 are needed
    (folded into the next layer's bias via -m*rowsum(W)), plus ONE rsqrt
    for LN3 at the very end.
  * Linear evacuations run on the scalar (ACT) engine as
    relu(psum + bias_col) with accum_out collecting the per-partition sums
    for the LN stats; the L3 sum-of-squares pass runs on the vector engine.
  * Input DMA is descriptor-generation-bound (~50ns/descriptor/queue, one
    descriptor per SBUF partition row), so inputs are packed into few
    tensors with long rows, split across the two HWDGE queues by partition
    halves; selection chunks 1 and 3 ride the gpsimd SWDGE queue as uint8
    with on-the-fly cast to fp16.
"""

import sys
import os

for _p in ("/opt/trn_rl_repo",):
    if _p not in sys.path and os.path.isdir(_p):
        sys.path.insert(0, _p)

import numpy as np

N = 84
H = 64
M = N * (N - 1) // 2  # 3486
F = M // 2            # 1743 per half
F_PAD = 1744          # even row stride for fp16 tiles
EPS = 1e-5
CHUNKS = [(0, 512), (512, 512), (1024, 512), (1536, 207)]
CORDER = [0, 1, 2, 3]      # processing order (arrival-matched; narrow last)
TCH = CHUNKS               # tail-layer grid
NSH = 3               # chunks 0..NSH-1 use the shared-j 3-section format
# Newton rsqrt seed y0 = RA/v + RB + RC*v (16.6% max rel err on [0.04, 6])
RA, RB, RC = 0.19709184, 0.90519586, -0.09958437
NR_ITERS = 2
PKW = 600    # pkA columns
WTW = 192    # wtsA columns (w1T | w2Tp | w3Tp)
C1W = 646    # consts1 columns


def _pair_maps():
    """Column -> (i, j) maps per half.  Columns 0:1722 have ju identical
    across halves; the 21 mixed leftovers sit at the end."""
    iu = [[], []]
    ju = [[], []]
    for j in range(1, N):
        k = j // 2
        for t in range(k):
            iu[0].append(t)
            ju[0].append(j)
            iu[1].append(k + t)
            ju[1].append(j)
    left = [j for j in range(1, N) if j % 2 == 1]
    for m in range(0, len(left), 2):
        ja, jb = left[m], left[m + 1]
        iu[0].append(ja - 1)
        ju[0].append(ja)
        iu[1].append(jb - 1)
        ju[1].append(jb)
    return (np.array(iu[0]), np.array(ju[0]),
            np.array(iu[1]), np.array(ju[1]))


_IU0, _JU0, _IU1, _JU1 = _pair_maps()

_prog_cache = {}


def _build_program(dbg=False):
    import concourse.bacc as bacc
    import concourse.mybir as mybir
    from concourse import tile

    f32 = mybir.dt.float32
    f16 = mybir.dt.float16
    u8 = mybir.dt.uint8
    AF = mybir.ActivationFunctionType
    OP = mybir.AluOpType

    nc = bacc.Bacc("TRN2", target_bir_lowering=False, debug=False)

    pkA_d = nc.dram_tensor("pkA", [N, PKW], f16, kind="ExternalInput")
    wts_d = nc.dram_tensor("wtsA", [H, WTW], f16, kind="ExternalInput")
    c1_d = nc.dram_tensor("consts1", [1, C1W], f32, kind="ExternalInput")
    scmb_d = []
    for ci, (c0, cw) in enumerate(CHUNKS):
        ns = 3 if ci < NSH else 4
        scmb_d.append(nc.dram_tensor(f"scmb{ci}", [N, ns * cw], f16,
                                     kind="ExternalInput"))
    out_d = nc.dram_tensor("o", [2, F], f32, kind="ExternalOutput")
    dbg_d = {}
    if dbg:
        for nm, shp in [("h", [128, F]), ("y1", [128, F]), ("y2", [128, F]),
                        ("y3", [128, F]), ("ST1", [128, 4]), ("ST2", [128, 4]),
                        ("ST3", [128, 8])]:
            dbg_d[nm] = nc.dram_tensor("dbg_" + nm, shp, f32,
                                       kind="ExternalOutput")

    with tile.TileContext(nc) as tc:
        with (
            tc.tile_pool(name="cons", bufs=1) as cons,
            tc.tile_pool(name="spool", bufs=1) as spool,
            tc.tile_pool(name="big", bufs=1) as big,
            tc.tile_pool(name="scr", bufs=2) as scr,
            tc.tile_pool(name="nrp", bufs=1) as nrp,
            tc.tile_pool(name="psrz", bufs=2, space="PSUM") as psrz,
            tc.tile_pool(name="psnb", bufs=1, space="PSUM") as psnb,
            tc.tile_pool(name="psm", bufs=1, space="PSUM") as psm,
        ):
            # ---- persistent SBUF tiles ----
            pk = cons.tile([N, PKW], f16, tag="pk")
            xT_aug = pk[0:H + 1, 0:84]       # rows 0:64 x.T, row 64 ones
            x_t = pk[0:N, 84:148]            # x (half of x2dup)
            x2dup = pk[0:N, 84:212]          # x | x
            wih_aug = pk[0:H + 1, 212:404]   # rows 0:64 W_ih.T, row 64 b_ih
            whh_aug = pk[0:H + 1, 404:596]
            # weight blocks duplicated on both partition halves so the
            # per-half matmuls can sit at PE quadrants (0,0)/(64,64)
            wts = cons.tile([128, WTW], f16, tag="wts")
            w1T = (wts[0:64, 0:64], wts[64:128, 0:64])
            w2T = (wts[0:64, 64:128], wts[64:128, 64:128])
            w3T = (wts[0:64, 128:192], wts[64:128, 128:192])
            w4bd = cons.tile([128, 2], f16, tag="w4bd")

            c1 = cons.tile([1, C1W], f32, tag="c1")
            w4row = c1[:, 384:386]
            b4row = c1[:, 386:388]
            ones2row = c1[:, 388:390]

            LA = cons.tile([N, 3 * H], f16, tag="LA")
            LB2 = cons.tile([N, 6 * H], f16, tag="LB2")  # r|r|z|z|n|n
            ones_col = cons.tile([128, 1], f32, tag="ones_col")
            onecell = ones_col[0:1, 0:1]
            b1col = cons.tile([128, 1], f32, tag="b1col")
            c2col = cons.tile([128, 1], f32, tag="c2col")
            c3col = cons.tile([128, 1], f32, tag="c3col")

            scmb_t = []
            for ci, (c0, cw) in enumerate(CHUNKS):
                ns = 3 if ci < NSH else 4
                st = spool.tile([N, ns * cw], f16, tag=f"scmb{ci}",
                                name=f"scmb{ci}")
                scmb_t.append(st)

            y1T = big.tile([128, F_PAD], f16, tag="y1T")
            y2T = big.tile([128, F_PAD], f16, tag="y2T")
            y3T = big.tile([128, F_PAD], f16, tag="y3T")
            oT = big.tile([2, F], f32, tag="oT")
            ST1 = big.tile([128, 4], f32, tag="ST1")
            ST2 = big.tile([128, 4], f32, tag="ST2")
            ST3 = big.tile([128, 8], f32, tag="ST3")

            # ---- input DMAs: HW queues carry only the selection chunks
            # (descriptor-generation-bound: ~2.1us per 42-row half); pk and
            # the small tensors ride the SWDGE queue whose generation is
            # nearly free.
            for ci in range(4):
                nc.sync.dma_start(scmb_t[ci][0:42, :],
                                  scmb_d[ci].ap()[0:42, :])
                nc.scalar.dma_start(scmb_t[ci][42:N, :],
                                    scmb_d[ci].ap()[42:N, :])
            nc.gpsimd.dma_start(pk[:], pkA_d.ap())
            nc.gpsimd.dma_start(c1[:], c1_d.ap())
            nc.gpsimd.dma_start(wts[0:64, :], wts_d.ap())
            nc.gpsimd.dma_start(wts[64:128, :], wts_d.ap())

            # table preload: dummy sigmoid on a memset cell (no DMA dep)
            wsrc = nrp.tile([1, 1], f32, tag="wsrc")
            nc.vector.memset(wsrc[:], 0.0)
            warm = nrp.tile([1, 1], f32, tag="warm")
            nc.scalar.activation(warm[:], wsrc[:], AF.Sigmoid)

            nc.vector.memset(ones_col[:], 1.0)

            # PE p-state ramp warmers: keep the PE array streaming during the
            # input-DMA window so the clock is at full speed when the real
            # work lands.  Harmless matmuls on a zeroed scratch tile.
            wsc = cons.tile([128, 512], f16, tag="wsc")
            nc.vector.memset(wsc[:], 0.0)
            p_warm = psrz.tile([128, 1024], f32, tag="p_rz", name="p_warm")
            for wk in range(8):
                nc.tensor.matmul(p_warm[0:64, 0:512], wsc[:, 0:64], wsc[:],
                                 start=True, stop=True,
                                 skip_group_check=True)

            # ---- A = [x|1]@[W_ih.T;b_ih], B likewise (PE-first: critical) --
            pA0 = psm.tile([N, 3 * H], f32, tag="p_l", padded_shape=[N, 512],
                           name="pA0")
            nc.tensor.matmul(pA0[:], xT_aug, wih_aug, start=True, stop=True)
            nc.vector.tensor_scalar(LA[:], pA0[:], 1.0, None, OP.mult)
            pB0 = psnb.tile([N, 3 * H], f32, tag="p_An",
                            padded_shape=[128, 512], name="pB0")
            nc.tensor.matmul(pB0[:], xT_aug, whh_aug, start=True, stop=True)
            src_b = pB0[:].rearrange("p (g o d) -> p g o d", g=3,
                                     o=1).broadcast_to((N, 3, 2, 64))
            dst_b = LB2[:].rearrange("p (g o d) -> p g o d", g=3, o=2)
            nc.vector.tensor_scalar(dst_b, src_b, 1.0, None, OP.mult)

            # b1col transpose (needed by first L1 evacuation)
            p_b1 = psm.tile([128, 1], f32, tag="p_l", padded_shape=[128, 512],
                            name="p_b1")
            nc.tensor.matmul(p_b1[:], c1[:, 0:128], onecell, start=True,
                             stop=True)
            nc.vector.tensor_scalar(b1col[:], p_b1[:], 1.0, None, OP.mult)
            # ---- GRU + L1, chunk by chunk (emission software-pipelined) ----
            PO = (slice(0, 64), slice(64, 128))
            TP = ((0, 0), (0, 64))

            def gru_chunk_mm(ci):
                c0, cw = CHUNKS[ci]
                st = scmb_t[ci]
                p_rz = psrz.tile([128, 1024], f32, tag="p_rz")
                p_An = psnb.tile([128, cw], f32, tag="p_An",
                                 padded_shape=[128, 512])
                p_Bn = psnb.tile([128, cw], f32, tag="p_Bn",
                                 padded_shape=[128, 512])
                p_x2 = psnb.tile([128, cw], f32, tag="p_x2",
                                 padded_shape=[128, 512])

                if ci < NSH:
                    siu = (st[:, 0:cw], st[:, cw:2 * cw])
                    sju = st[:, 2 * cw:3 * cw]

                    def rz_group(g, Lsl):
                        for hi in range(2):
                            nc.tensor.matmul(p_rz[PO[hi], 512 * g:512 * g + cw],
                                             LA[:, Lsl], siu[hi],
                                             start=True, stop=False,
                                             tile_position=TP[hi],
                                             skip_group_check=True)
                        nc.tensor.matmul(p_rz[:, 512 * g:512 * g + cw],
                                         LB2[:, 128 * g:128 * g + 128], sju,
                                         start=False, stop=True,
                                         skip_group_check=True)

                    rz_group(0, slice(0, 64))           # r gate first
                    nc.tensor.matmul(p_Bn[:], LB2[:, 256:384], sju,
                                     start=True, stop=True,
                                     skip_group_check=True)
                    for hi in range(2):
                        nc.tensor.matmul(p_An[PO[hi], :], LA[:, 128:192],
                                         siu[hi], start=True, stop=True,
                                         tile_position=TP[hi],
                                         skip_group_check=True)
                    rz_group(1, slice(64, 128))         # z gate
                    nc.tensor.matmul(p_x2[:], x2dup, sju, start=True,
                                     stop=True, skip_group_check=True)
                else:
                    siu = (st[:, 0:cw], st[:, 2 * cw:3 * cw])
                    sju = (st[:, cw:2 * cw], st[:, 3 * cw:4 * cw])

                    def rz_group4(g):
                        for hi in range(2):
                            nc.tensor.matmul(p_rz[PO[hi], 512 * g:512 * g + cw],
                                             LA[:, 64 * g:64 * g + 64],
                                             siu[hi],
                                             start=True, stop=False,
                                             tile_position=TP[hi],
                                             skip_group_check=True)
                            nc.tensor.matmul(p_rz[PO[hi], 512 * g:512 * g + cw],
                                             LB2[:, 128 * g:128 * g + 64],
                                             sju[hi],
                                             start=False, stop=True,
                                             tile_position=TP[hi],
                                             skip_group_check=True)

                    rz_group4(0)                        # r gate first
                    for hi in range(2):
                        nc.tensor.matmul(p_Bn[PO[hi], :], LB2[:, 256:320],
                                         sju[hi], start=True, stop=True,
                                         tile_position=TP[hi],
                                         skip_group_check=True)
                        nc.tensor.matmul(p_An[PO[hi], :], LA[:, 128:192],
                                         siu[hi], start=True, stop=True,
                                         tile_position=TP[hi],
                                         skip_group_check=True)
                    rz_group4(1)                        # z gate
                    for hi in range(2):
                        nc.tensor.matmul(p_x2[PO[hi], :], x_t, sju[hi],
                                         start=True, stop=True,
                                         tile_position=TP[hi],
                                         skip_group_check=True)
                return p_rz, p_An, p_Bn, p_x2

            def gru_chunk_ew(ci, p_rz, p_An, p_Bn, p_x2, last=False):
                c0, cw = CHUNKS[ci]
                csl = slice(c0, c0 + cw)
                rz_c = scr.tile([128, 2 * cw], f16, tag="rz", name="rz")
                s_c = scr.tile([128, cw], f16, tag="s")
                s2_c = scr.tile([128, cw], f16, tag="s2")
                nn_c = scr.tile([128, cw], f16, tag="nn")
                zx2_c = scr.tile([128, cw], f16, tag="zx2")
                q_c = scr.tile([128, cw], f16, tag="q")
                h_c = scr.tile([128, cw], f16, tag="h")

                r_sl = rz_c[:, 0:cw]
                z_sl = rz_c[:, cw:2 * cw]

                nc.scalar.activation(r_sl, p_rz[:, 0:cw], AF.Sigmoid)
                nc.scalar.activation(z_sl, p_rz[:, 512:512 + cw], AF.Sigmoid)
                nc.vector.tensor_tensor(s_c[:], r_sl, p_Bn[:], OP.mult)
                nc.vector.tensor_tensor(s2_c[:], s_c[:], p_An[:], OP.add)
                nc.scalar.activation(nn_c[:], s2_c[:], AF.Tanh)
                nc.vector.scalar_tensor_tensor(q_c[:], z_sl, 1.0, nn_c[:],
                                               OP.subtract, OP.mult)
                nc.vector.tensor_tensor(zx2_c[:], z_sl, p_x2[:], OP.mult)
                heng = nc.vector if last else nc.gpsimd
                heng.tensor_tensor(h_c[:], zx2_c[:], q_c[:], OP.subtract)

                p_l1 = psm.tile([128, cw], f32, tag="p_l",
                                padded_shape=[128, 512], name=f"p_l1{ci}")
                for hi in range(2):
                    nc.tensor.matmul(p_l1[PO[hi], :], w1T[hi], h_c[PO[hi], :],
                                     start=True, stop=True,
                                     tile_position=(64 * hi, 64 * hi),
                                     skip_group_check=True)
                nc.scalar.activation(y1T[:, csl], p_l1[:], AF.Relu,
                                     bias=b1col[:],
                                     accum_out=ST1[:, ci:ci + 1])
                if dbg:
                    dbg_h = scr.tile([128, cw], f32, tag="dbgh", name="dbg_h")
                    nc.vector.tensor_scalar(dbg_h[:], h_c[:], 1.0, None,
                                            OP.mult)
                    nc.sync.dma_start(dbg_d["h"].ap()[:, csl], dbg_h[:])

            for k, ci in enumerate(CORDER):
                ps = gru_chunk_mm(ci)
                gru_chunk_ew(ci, *ps, last=(k >= len(CORDER) - 2))

            zcol = cons.tile([128, 1], f32, tag="zcol")
            nc.vector.memset(zcol[:], 0.0)

            def lpool(ci, shape, nm):
                """Rotate tail-layer PSUM over 4 slots: psm, psnb:p_x2,
                and the two (now free) p_rz buffers."""
                if ci % 4 == 0:
                    return psm.tile(shape, f32, tag="p_l",
                                    padded_shape=[shape[0], 512], name=nm)
                if ci % 4 == 1:
                    return psnb.tile(shape, f32, tag="p_x2",
                                     padded_shape=[128, 512], name=nm)
                t = psrz.tile([128, 1024], f32, tag="p_rz", name=nm)
                return t[0:shape[0], 0:shape[1]]

            def evac_relu(ci, dst, src, bias, STt):
                """relu(src + bias) -> dst, accum -> STt; chunks 0/1 on
                ACT, 2/3 on DVE to halve the evacuation wall."""
                if ci % 4 in (0, 1):
                    nc.scalar.activation(dst, src, AF.Relu, bias=bias,
                                         accum_out=STt)
                else:
                    nc.vector.scalar_tensor_tensor(
                        dst, src, bias,
                        zcol.broadcast_to((128, src.shape[1])),
                        OP.add, OP.max, accum_out=STt)

            # ---- L2 matmuls (overlap the GRU drain + chain1) ----
            p_l2 = {}
            for ci, (c0, cw) in enumerate(TCH):
                p_l2[ci] = lpool(ci, [128, cw], f"p_l2{ci}")
                for hi in range(2):
                    nc.tensor.matmul(p_l2[ci][PO[hi], :], w2T[hi],
                                     y1T[PO[hi], c0:c0 + cw], start=True,
                                     stop=True,
                                     tile_position=(64 * hi, 64 * hi),
                                     skip_group_check=True)

            # ---- chain1: m1 only -> c2col = -m1*rowsum(W2) ----
            p_s1 = psnb.tile([1, 4], f32, tag="p_An",
                             padded_shape=[128, 512], name="p_s1")
            nc.tensor.matmul(p_s1[:], ones_col[:], ST1[:], start=True,
                             stop=True)
            s1 = nrp.tile([1, 1], f32, tag="s1")
            nc.vector.tensor_reduce(s1[:], p_s1[:], axis=mybir.AxisListType.X,
                                    op=OP.add)
            p_c2 = psnb.tile([128, 1], f32, tag="p_Bn",
                             padded_shape=[128, 512], name="p_c2")
            nc.tensor.matmul(p_c2[:], c1[:, 128:256], s1[:], start=True,
                             stop=True)
            nc.vector.tensor_scalar(c2col[:], p_c2[:], 1.0, None, OP.mult)

            # ---- L2 evacuations ----
            for ci, (c0, cw) in enumerate(TCH):
                csl = slice(c0, c0 + cw)
                evac_relu(ci, y2T[:, csl], p_l2[ci][:], c2col[:],
                          ST2[:, ci:ci + 1])

            # ---- L3 matmuls (overlap chain2) ----
            p_l3 = {}
            for ci, (c0, cw) in enumerate(TCH):
                p_l3[ci] = lpool(ci, [128, cw], f"p_l3{ci}")
                for hi in range(2):
                    nc.tensor.matmul(p_l3[ci][PO[hi], :], w3T[hi],
                                     y2T[PO[hi], c0:c0 + cw], start=True,
                                     stop=True,
                                     tile_position=(64 * hi, 64 * hi),
                                     skip_group_check=True)

            # ---- chain2: m^2 only -> c3col ----
            p_s2 = psnb.tile([1, 4], f32, tag="p_An",
                             padded_shape=[128, 512], name="p_s2")
            nc.tensor.matmul(p_s2[:], ones_col[:], ST2[:], start=True,
                             stop=True)
            s2s = nrp.tile([1, 1], f32, tag="s2s")
            nc.vector.tensor_reduce(s2s[:], p_s2[:], axis=mybir.AxisListType.X,
                                    op=OP.add)
            p_c3 = psnb.tile([128, 1], f32, tag="p_Bn",
                             padded_shape=[128, 512], name="p_c3")
            nc.tensor.matmul(p_c3[:], c1[:, 256:384], s2s[:], start=True,
                             stop=True)
            nc.vector.tensor_scalar(c3col[:], p_c3[:], 1.0, None, OP.mult)

            # w4 block-diag columns (needed before L4)
            p_w4a = psnb.tile([128, 1], f32, tag="p_An",
                              padded_shape=[128, 512], name="p_w4a")
            nc.tensor.matmul(p_w4a[:], c1[:, 390:518], onecell, start=True,
                             stop=True)
            nc.vector.tensor_scalar(w4bd[:, 0:1], p_w4a[:], 1.0, None, OP.mult)
            p_w4b = psnb.tile([128, 1], f32, tag="p_Bn",
                              padded_shape=[128, 512], name="p_w4b")
            nc.tensor.matmul(p_w4b[:], c1[:, 518:646], onecell, start=True,
                             stop=True)
            nc.vector.tensor_scalar(w4bd[:, 1:2], p_w4b[:], 1.0, None, OP.mult)

            # ---- L3 evacuations (+ sum-of-squares for LN3 stats) ----
            for ci, (c0, cw) in enumerate(TCH):
                csl = slice(c0, c0 + cw)
                evac_relu(ci, y3T[:, csl], p_l3[ci][:], c3col[:],
                          ST3[:, ci:ci + 1])
                dump = scr.tile([128, cw], f16, tag="dump", name="dump")
                if ci % 4 in (0, 1):
                    nc.vector.scalar_tensor_tensor(
                        dump[:], y3T[:, csl], 0.0, y3T[:, csl], OP.add,
                        OP.mult, accum_out=ST3[:, 4 + ci:5 + ci])
                else:
                    nc.scalar.activation(dump[:], y3T[:, csl], AF.Square,
                                         accum_out=ST3[:, 4 + ci:5 + ci])

            # ---- chain3: mean+var -> a3 (single rsqrt via recip+Newton) ----
            p_s3 = psnb.tile([1, 8], f32, tag="p_An",
                             padded_shape=[128, 512], name="p_s3")
            nc.tensor.matmul(p_s3[:], ones_col[:], ST3[:], start=True,
                             stop=True)

            # L4 matmuls for all chunks run during the scalar chain
            p_l4 = {}
            for ci, (c0, cw) in enumerate(TCH):
                p_l4[ci] = lpool(ci, [2, cw], f"p_l4{ci}")
                nc.tensor.matmul(p_l4[ci][:], w4bd[:],
                                 y3T[:, c0:c0 + cw], start=True, stop=True)

            sums = nrp.tile([1, 2], f32, tag="sums")
            nc.vector.tensor_reduce(
                sums[:], p_s3[:].rearrange("p (a b) -> p a b", a=2),
                axis=mybir.AxisListType.X, op=OP.add)
            mq = nrp.tile([1, 2], f32, tag="mq")
            nc.vector.tensor_scalar(mq[:], sums[:], 1.0 / (M * 32.0), None,
                                    OP.mult)
            m2 = nrp.tile([1, 1], f32, tag="m2")
            nc.vector.tensor_scalar(m2[:], mq[:, 0:1], mq[:, 0:1], None,
                                    OP.mult)
            v_t = nrp.tile([1, 1], f32, tag="v")
            nc.vector.scalar_tensor_tensor(v_t[:], m2[:], -1.0, mq[:, 1:2],
                                           OP.mult, OP.add)
            nc.vector.tensor_scalar(v_t[:], v_t[:], 1.0, EPS, OP.mult, OP.add)
            vqs = []
            for k in range(NR_ITERS):
                vq = nrp.tile([1, 1], f32, tag=f"vq{k}", name=f"vq{k}")
                nc.vector.tensor_scalar(vq[:], v_t[:], 0.25 ** k, None,
                                        OP.mult)
                vqs.append(vq)
            rv = nrp.tile([1, 1], f32, tag="rv")
            nc.vector.reciprocal(rv[:], v_t[:])
            t1 = nrp.tile([1, 1], f32, tag="t1")
            nc.vector.tensor_scalar(t1[:], v_t[:], RC, RB, OP.mult, OP.add)
            w_t = nrp.tile([1, 1], f32, tag="w")
            nc.vector.scalar_tensor_tensor(w_t[:], rv[:], RA, t1[:],
                                           OP.mult, OP.add)
            t_t = nrp.tile([1, 1], f32, tag="t")
            for k in range(NR_ITERS):
                nc.vector.tensor_scalar(t_t[:], w_t[:], w_t[:], vqs[k][:],
                                        OP.mult, OP.mult)
                nc.vector.scalar_tensor_tensor(w_t[:], t_t[:], 3.0, w_t[:],
                                               OP.subtract, OP.mult)
            G = nrp.tile([1, 1], f32, tag="G")
            nc.vector.tensor_scalar(G[:], w_t[:], (-0.5) ** NR_ITERS, None,
                                    OP.mult)
            # scale4 = a3 on both partitions; bias4 = -a3*m3*rowsum(W4) + b4
            A4 = nrp.tile([1, 1], f32, tag="A4")
            nc.vector.tensor_scalar(A4[:], mq[:, 0:1], G[:], -1.0,
                                    OP.mult, OP.mult)
            p_s4 = psnb.tile([2, 2], f32, tag="p_Bn",
                             padded_shape=[128, 512], name="p_s4")
            nc.tensor.matmul(p_s4[:, 0:1], ones2row, G[:], start=True,
                             stop=True)
            nc.tensor.matmul(p_s4[:, 1:2], w4row, A4[:], start=True,
                             stop=False)
            nc.tensor.matmul(p_s4[:, 1:2], b4row, onecell, start=False,
                             stop=True)
            sc4 = nrp.tile([2, 2], f32, tag="sc4")
            nc.vector.tensor_scalar(sc4[:], p_s4[:], 1.0, None, OP.mult)
            scale4 = sc4[:, 0:1]
            bias4 = sc4[:, 1:2]

            # ---- L4 + sigmoid + output ----
            for ci, (c0, cw) in enumerate(TCH):
                csl = slice(c0, c0 + cw)
                nc.scalar.activation(oT[:, csl], p_l4[ci][:], AF.Sigmoid,
                                     bias=bias4, scale=scale4)
                nc.sync.dma_start(out_d.ap()[:, csl], oT[:, csl])
            if dbg:
                for nm, t in [("y1", y1T), ("y2", y2T), ("y3", y3T)]:
                    dt_ = big.tile([128, F], f32, tag="dbgy" + nm,
                                   name="dbgy" + nm)
                    nc.vector.tensor_scalar(dt_[:], t[0:128, 0:F], 1.0, None,
                                            OP.mult)
                    nc.sync.dma_start(dbg_d[nm].ap(), dt_[:])
                nc.sync.dma_start(dbg_d["ST1"].ap(), ST1[:])
                nc.sync.dma_start(dbg_d["ST2"].ap(), ST2[:])
                nc.sync.dma_start(dbg_d["ST3"].ap(), ST3[:])

    nc.compile()
    return nc


def _host_inputs(inputs):
    """Build the device input map from the raw model inputs."""
    x = np.ascontiguousarray(inputs["x"], np.float32)
    W_ih = np.asarray(inputs["W_ih"], np.float32)
    W_hh = np.asarray(inputs["W_hh"], np.float32)
    b_ih = np.asarray(inputs["b_ih"], np.float32)
    b_hh = np.asarray(inputs["b_hh"], np.float32)
    W1 = np.asarray(inputs["W1"], np.float32)
    b1 = np.asarray(inputs["b1"], np.float32)
    W2 = np.asarray(inputs["W2"], np.float32)
    W3 = np.asarray(inputs["W3"], np.float32)
    W4 = np.asarray(inputs["W4"], np.float32)
    b4 = np.asarray(inputs["b4"], np.float32)
    f16 = np.float16

    pk = np.zeros((N, PKW), f16)
    pk[0:H, 0:84] = x.T
    pk[H, 0:84] = 1.0
    pk[0:N, 84:148] = x
    pk[0:N, 148:212] = x
    pk[0:H, 212:404] = W_ih.T
    pk[H, 212:404] = b_ih
    pk[0:H, 404:596] = W_hh.T
    pk[H, 404:596] = b_hh

    wt = np.zeros((H, WTW), f16)
    wt[0:64, 0:64] = W1.T
    wt[0:64, 64:96] = W2.T          # [64, 32]; cols 96:128 stay 0
    wt[0:32, 128:160] = W3.T        # [32, 32]; rest 0

    c1 = np.zeros((1, C1W), np.float32)
    c1[0, 0:64] = b1
    c1[0, 64:128] = b1
    c1[0, 128:160] = W2.sum(1) * (-1.0 / (M * 64.0))
    c1[0, 192:224] = W2.sum(1) * (-1.0 / (M * 64.0))
    c1[0, 256:288] = W3.sum(1) * (-1.0 / (M * 32.0))
    c1[0, 320:352] = W3.sum(1) * (-1.0 / (M * 32.0))
    c1[0, 384:386] = W4.sum(1)
    c1[0, 386:388] = b4
    c1[0, 388:390] = 1.0
    c1[0, 390:422] = W4[0, :]       # w4bd col 0, partitions 0:32
    c1[0, 582:614] = W4[0, :]       # w4bd col 1, partitions 64:96

    def onehot(idx):
        S = np.zeros((N, F), f16)
        S[idx, np.arange(F)] = 1.0
        return S

    siu0, sju0 = onehot(_IU0), onehot(_JU0)
    siu1, sju1 = onehot(_IU1), onehot(_JU1)

    out = {"pkA": pk, "wtsA": wt, "consts1": c1}
    for ci, (c0, cw) in enumerate(CHUNKS):
        sl = slice(c0, c0 + cw)
        if ci < NSH:
            sc = np.concatenate([siu0[:, sl], siu1[:, sl], sju0[:, sl]],
                                axis=1)
        else:
            sc = np.concatenate([siu0[:, sl], sju0[:, sl],
                                 siu1[:, sl], sju1[:, sl]], axis=1)
        out[f"scmb{ci}"] = np.ascontiguousarray(sc)
    return out


def _assemble(o_packed):
    A = np.zeros((N, N), np.float32)
    A[_IU0, _JU0] = o_packed[0]
    A[_IU1, _JU1] = o_packed[1]
    return A + A.T


def _supported(inputs):
    """The fast path folds away identity LayerNorm affines and requires
    b2 == b3 == 0 (true for the canonical setup_inputs)."""
    for g in ("g1", "g2", "g3"):
        if g in inputs and not np.all(np.asarray(inputs[g]) == 1.0):
            return False
    for b in ("be1", "be2", "be3", "b2", "b3"):
        if b in inputs and not np.all(np.asarray(inputs[b]) == 0.0):
            return False
    return True


def _numpy_reference(inputs):
    """Generic fallback (non-identity LayerNorm affine params only)."""
    x = np.asarray(inputs["x"], np.float64)
    iu, ju = np.triu_indices(N, k=1)
    gi = x[iu] @ np.asarray(inputs["W_ih"]).T + np.asarray(inputs["b_ih"])
    gh = x[ju] @ np.asarray(inputs["W_hh"]).T + np.asarray(inputs["b_hh"])
    i_r, i_z, i_n = np.split(gi, 3, 1)
    h_r, h_z, h_n = np.split(gh, 3, 1)
    r = 1 / (1 + np.exp(-(i_r + h_r)))
    z = 1 / (1 + np.exp(-(i_z + h_z)))
    nn_ = np.tanh(i_n + r * h_n)
    h = (1 - z) * nn_ + z * x[ju]

    def ln(y, g, b):
        m = y.mean()
        v = ((y - m) ** 2).mean()
        return (y - m) / np.sqrt(v + EPS) * np.asarray(g) + np.asarray(b)

    h = ln(np.maximum(h @ np.asarray(inputs["W1"]).T + np.asarray(inputs["b1"]), 0),
           inputs["g1"], inputs["be1"])
    h = ln(np.maximum(h @ np.asarray(inputs["W2"]).T + np.asarray(inputs["b2"]), 0),
           inputs["g2"], inputs["be2"])
    h = ln(np.maximum(h @ np.asarray(inputs["W3"]).T + np.asarray(inputs["b3"]), 0),
           inputs["g3"], inputs["be3"])
    o = 1 / (1 + np.exp(-(h @ np.asarray(inputs["W4"]).T + np.asarray(inputs["b4"]))))
    A = np.zeros((N, N), np.float32)
    A[iu, ju] = o[:, 0]
    return A + A.T


def kernel(**inputs):
    if not _supported(inputs):
        return _numpy_reference(inputs)

    if "nc" not in _prog_cache:
        _prog_cache["nc"] = _build_program()
    nc = _prog_cache["nc"]

    from concourse.bass_utils import run_bass_kernel_spmd

    in_map = _host_inputs(inputs)
    res = run_bass_kernel_spmd(nc, [in_map], core_ids=[0])
    return _assemble(res.results[0]["o"])


if __name__ == "__main__":
    sys.path.insert(0, os.path.dirname(os.path.abspath(__file__)))
    import jax
    jax.config.update("jax_platforms", "cpu")
    import reference

    ins = {k: np.asarray(v) for k, v in reference.setup_inputs().items()}
    expected = np.asarray(reference.reference(**ins))
    got = kernel(**ins)
    err = np.abs(got - expected).max()
    print("absmax err:", err, "rel:", err / np.abs(expected).max())
